# revision 13
# baseline (speedup 1.0000x reference)
"""MixHop GNN (2-hop GCN propagation + MLP head) on 8 Trainium2 NeuronCores.

Strategy (node-sharded by dst, streaming — no on-device gather):
  norm factorization:  norm = dis[src]*dis[dst] ->  hop(v) = dis * S(dis * v)
  with S the plain scatter-sum over edges; self loops handled as a direct
  per-node add in the tail (never materialized as edges).

  Between launches the HOST materializes the per-edge value stream
  v_e = u[src_e] (fp8) in dst-grouped k-tile order, plus a STATIC fp8
  one-hot stream for the scatter matrices (built once, reused by both
  hops).  Each core consumes both streams SEQUENTIALLY with fat DMA
  descriptors; the scatter-sum runs on the Tensor engine as
      psum[dst_tile 128, H] += OneHotT(fp8)^T @ v_tile(fp8)
  two k-tiles per instruction (fp8 DoubleRow).  PSUM accumulation is
  in-order => no scatter races.  No SWDGE descriptor generation, no
  random-access DMA, no on-chip one-hot build.  L3 interleaves the dense
  MLP head + per-block log-softmax into the hop so it overlaps the
  stream DMA.

  3 SPMD launches over 8 cores:
    L1: h = relu(x@w1+b1), u0 = dis*h                (row shard per core)
    L2: hop1 over u0-stream -> h1, u1 shards
    L3: hop2 over u1-stream + fused MLP tail -> log_softmax logits shard
"""

import numpy as np
import ml_dtypes

import concourse.bacc as bacc
import concourse.bass as bass
import concourse.tile as tile
from concourse import mybir
from concourse.bass_utils import run_bass_kernel_spmd

BF16 = ml_dtypes.bfloat16
FP8 = ml_dtypes.float8_e4m3
AF = mybir.ActivationFunctionType
ALU = mybir.AluOpType
DR = mybir.MatmulPerfMode.DoubleRow

N, E, F_IN, H, C = 100000, 1600000, 256, 64, 40
NCORE = 8
NSH = N // NCORE            # 12500 nodes per core
NT = (NSH + 127) // 128     # 98 dst tiles per core
NTP = NT * 128              # 12544 padded rows
VB = 96                     # k-tiles per stream staging block
TB = 4                      # dst tiles per dense-tail block in L3

_cache = {}
_last_runs = []


# --------------------------------------------------------------------------
# host-side graph partitioning / padding plan
# --------------------------------------------------------------------------

def _prep_graph(edge_index):
    src = np.asarray(edge_index[0], dtype=np.int64)
    dst = np.asarray(edge_index[1], dtype=np.int64)
    deg = (np.bincount(dst, minlength=N) + 1).astype(np.float32)  # + self loop
    dis = (1.0 / np.sqrt(deg)).astype(np.float32)

    per_core = []
    cnts = np.zeros((NCORE, NT), np.int64)
    for c in range(NCORE):
        sel = (dst // NSH) == c
        s_g = src[sel]
        d_l = (dst[sel] - c * NSH).astype(np.int64)
        t_id = d_l // 128
        order = np.argsort(t_id, kind="stable")
        s_g, d_l, t_id = s_g[order], d_l[order], t_id[order]
        cnts[c] = np.bincount(t_id, minlength=NT)
        per_core.append((s_g, d_l, t_id))

    nkt_t = np.ceil(cnts.max(axis=0) / 128.0).astype(np.int64)  # k-tiles/seg
    nkt_t = np.maximum(nkt_t, 1)
    off_t = np.zeros(NT + 1, np.int64)
    np.cumsum(nkt_t, out=off_t[1:])
    NKT = int(off_t[-1])

    srcs, ohs = [], []
    for c in range(NCORE):
        s_g, d_l, t_id = per_core[c]
        start = np.zeros(NT, np.int64)
        np.cumsum(cnts[c], out=start)
        start = np.concatenate([[0], start[:-1]])
        rank = np.arange(len(t_id)) - start[t_id]
        pos = off_t[t_id] * 128 + rank
        stream_src = np.zeros(NKT * 128, np.int64)
        stream_src[pos] = s_g
        srcs.append(stream_src)
        # static fp8 one-hot stream: row e has 1 at dst_local_in_tile
        oh = np.zeros((NKT * 128, 128), FP8)
        oh[pos, d_l - 128 * t_id] = 1
        ohs.append(np.ascontiguousarray(
            oh.reshape(NKT, 128, 128).transpose(1, 0, 2)
            .reshape(128, NKT * 128)))

    plan = dict(nkt_t=tuple(int(x) for x in nkt_t), NKT=NKT)
    return dis, srcs, ohs, plan


def _wrap_tiles(vec):
    """[NSH] -> [128, NT] with vec[t*128+p] at (p, t); pad zeros."""
    v = np.zeros(NTP, np.float32)
    v[:NSH] = vec
    return np.ascontiguousarray(v.reshape(NT, 128).T)


def _pm(a):
    """[rows<=NTP, F] -> partition-major [128, NT*F] (pad zeros)."""
    f = a.shape[1]
    v = np.zeros((NTP, f), np.float32)
    v[:a.shape[0]] = a
    return np.ascontiguousarray(
        v.reshape(NT, 128, f).transpose(1, 0, 2).reshape(128, NT * f))


def _unpm(a, f):
    """[128, NT*F] -> [NSH, F]."""
    return np.ascontiguousarray(
        a.reshape(128, NT, f).transpose(1, 0, 2).reshape(NTP, f)[:NSH])


def _stream_pm(table, stream_src, nkt):
    """Gather table rows [N, F] by stream -> [128, nkt*F] partition-major."""
    f = table.shape[1]
    g = table[stream_src]                     # [nkt*128, F]
    return np.ascontiguousarray(
        g.reshape(nkt, 128, f).transpose(1, 0, 2).reshape(128, nkt * f))


# --------------------------------------------------------------------------
# launch 1: h = relu(x@w1+b1); u0 = dis*h
# --------------------------------------------------------------------------

def _build_L1():
    nc = bacc.Bacc(None, target_bir_lowering=False, debug=False)
    # xTd[p, t, s, q] = x[t*128+q, s*128+p] : DoubleRow k-pair packed
    xTd = nc.dram_tensor("xTd", [128, NT * 2 * 128], mybir.dt.float8e4,
                         kind="ExternalInput")
    w1d = nc.dram_tensor("w1d", [128, 2 * H], mybir.dt.float8e4,
                         kind="ExternalInput")
    b1r = nc.dram_tensor("b1r", [1, H], mybir.dt.bfloat16, kind="ExternalInput")
    disw = nc.dram_tensor("disw", [128, NT], mybir.dt.float32, kind="ExternalInput")
    h_o = nc.dram_tensor("h", [128, NT * H], mybir.dt.bfloat16, kind="ExternalOutput")
    u0_o = nc.dram_tensor("u0", [128, NT * H], mybir.dt.bfloat16, kind="ExternalOutput")

    with tile.TileContext(nc) as tc:
        with (
            tc.tile_pool(name="per", bufs=1) as per,
            tc.tile_pool(name="ps", bufs=4, space="PSUM") as ps,
        ):
            xt = per.tile([128, NT, 2, 128], mybir.dt.float8e4)
            w1t = per.tile([128, 2, H], mybir.dt.float8e4)
            b1t = per.tile([1, H], mybir.dt.bfloat16)
            ones = per.tile([1, 128], mybir.dt.bfloat16)
            dt = per.tile([128, NT], mybir.dt.float32)
            h_sb = per.tile([128, NT, H], mybir.dt.bfloat16)
            u0_sb = per.tile([128, NT, H], mybir.dt.bfloat16)
            nc.sync.dma_start(xt[:], xTd.rearrange("p (t s q) -> p t s q",
                                                   s=2, q=128))
            nc.sync.dma_start(w1t[:], w1d.rearrange("p (s f) -> p s f", f=H))
            nc.sync.dma_start(b1t[:], b1r[:])
            nc.sync.dma_start(dt[:], disw[:])
            nc.vector.memset(ones[:], 1.0)
            for t in range(NT):
                pt = ps.tile([128, H], mybir.dt.float32, tag="mm")
                nc.tensor.matmul(pt[:], xt[:, t, :, :], w1t[:],
                                 start=True, stop=False, perf_mode=DR)
                nc.tensor.matmul(pt[:], ones[:], b1t[:], start=False, stop=True)
                nc.scalar.activation(h_sb[:, t, :], pt[:], AF.Relu)
                nc.vector.tensor_scalar(u0_sb[:, t, :], pt[:], 0.0,
                                        dt[:, t:t + 1], ALU.max, ALU.mult)
            nc.sync.dma_start(h_o.rearrange("p (t f) -> p t f", f=H), h_sb[:])
            nc.sync.dma_start(u0_o.rearrange("p (t f) -> p t f", f=H), u0_sb[:])
    nc.compile()
    return nc


# --------------------------------------------------------------------------
# shared hop machinery: fp8 streams, DoubleRow matmuls, psum[128 dst, H]
# --------------------------------------------------------------------------

def _make_stream(nc, sb, plan, vst, ohst):
    NKT = plan["NKT"]
    vv = vst.rearrange("p (k f) -> p k f", f=H)
    ov = ohst.rearrange("p (k f) -> p k f", f=128)
    blk = {}

    def get_blk(kt):
        b0 = (kt // VB) * VB
        if b0 not in blk:
            nb = min(VB, NKT - b0)
            vb = sb.tile([128, nb, H], mybir.dt.float8e4, tag="vb", bufs=3,
                         name=f"vb_{b0}")
            ob = sb.tile([128, nb, 128], mybir.dt.float8e4, tag="ob", bufs=3,
                         name=f"ob_{b0}")
            nc.sync.dma_start(vb[:], vv[:, b0:b0 + nb, :])
            nc.sync.dma_start(ob[:], ov[:, b0:b0 + nb, :])
            blk[b0] = (vb, ob)
        return blk[b0], kt - b0

    return get_blk


def _hop_segs(nc, ps, plan, get_blk, seg_fn, post_seg=None):
    nkt_t = plan["nkt_t"]
    kt = 0
    for t in range(NT):
        nkt = nkt_t[t]
        hp = ps.tile([128, H], mybir.dt.float32, tag="hp", bufs=2,
                     name=f"hp_{t}")
        i = 0
        while i < nkt:
            (vb, ob), j = get_blk(kt)
            pair = (i + 1 < nkt) and (kt // VB == (kt + 1) // VB)
            if pair:
                nc.tensor.matmul(hp[:], ob[:, j:j + 2, :], vb[:, j:j + 2, :],
                                 start=(i == 0), stop=(i + 2 == nkt),
                                 perf_mode=DR)
                i += 2
                kt += 2
            else:
                nc.tensor.matmul(hp[:], ob[:, j, :], vb[:, j, :],
                                 start=(i == 0), stop=(i + 1 == nkt))
                i += 1
                kt += 1
        seg_fn(t, hp)
        if post_seg is not None:
            post_seg(t)
    assert kt == plan["NKT"]


# --------------------------------------------------------------------------
# launch 2: hop1 -> h1, u1
# --------------------------------------------------------------------------

def _build_L2(plan):
    NKT = plan["NKT"]
    nc = bacc.Bacc(None, target_bir_lowering=False, debug=False)
    vst = nc.dram_tensor("vst", [128, NKT * H], mybir.dt.float8e4, kind="ExternalInput")
    ohst = nc.dram_tensor("ohst", [128, NKT * 128], mybir.dt.float8e4, kind="ExternalInput")
    ow1 = nc.dram_tensor("ow1", [128, NT * H], mybir.dt.bfloat16, kind="ExternalInput")
    dtw = nc.dram_tensor("dtw", [128, NT], mybir.dt.float32, kind="ExternalInput")
    h1_o = nc.dram_tensor("h1", [128, NT * H], mybir.dt.bfloat16, kind="ExternalOutput")
    u1_o = nc.dram_tensor("u1", [128, NT * H], mybir.dt.bfloat16, kind="ExternalOutput")

    with tile.TileContext(nc) as tc:
        with (
            tc.tile_pool(name="per", bufs=1) as per,
            tc.tile_pool(name="sb", bufs=2) as sb,
            tc.tile_pool(name="ps", bufs=2, space="PSUM") as ps,
        ):
            get_blk = _make_stream(nc, sb, plan, vst, ohst)
            get_blk(0)  # prefetch first stream block before table loads
            ow1_t = per.tile([128, NT, H], mybir.dt.bfloat16)
            dt = per.tile([128, NT], mybir.dt.float32)
            h1_sb = per.tile([128, NT, H], mybir.dt.bfloat16)
            u1_sb = per.tile([128, NT, H], mybir.dt.bfloat16)
            nc.sync.dma_start(dt[:], dtw[:])
            nc.sync.dma_start(ow1_t[:], ow1.rearrange("p (t f) -> p t f", f=H))

            def seg(t, hp):
                # h1 = dis*psum + ow1 ; u1 = dis*h1
                nc.vector.scalar_tensor_tensor(
                    h1_sb[:, t, :], hp[:], dt[:, t:t + 1], ow1_t[:, t, :],
                    ALU.mult, ALU.add)
                nc.vector.tensor_scalar(
                    u1_sb[:, t, :], h1_sb[:, t, :], dt[:, t:t + 1], None,
                    ALU.mult)

            _hop_segs(nc, ps, plan, get_blk, seg)
            nc.sync.dma_start(h1_o.rearrange("p (t f) -> p t f", f=H), h1_sb[:])
            nc.sync.dma_start(u1_o.rearrange("p (t f) -> p t f", f=H), u1_sb[:])
    nc.compile()
    return nc


# --------------------------------------------------------------------------
# launch 3: hop2 with fused dense MLP tail -> log_softmax logits
# --------------------------------------------------------------------------

def _build_L3(plan):
    NKT = plan["NKT"]
    nc = bacc.Bacc(None, target_bir_lowering=False, debug=False)
    vst = nc.dram_tensor("vst", [128, NKT * H], mybir.dt.float8e4, kind="ExternalInput")
    ohst = nc.dram_tensor("ohst", [128, NKT * 128], mybir.dt.float8e4, kind="ExternalInput")
    hT = nc.dram_tensor("hT", [H, NTP], mybir.dt.bfloat16, kind="ExternalInput")
    h1T = nc.dram_tensor("h1T", [H, NTP], mybir.dt.bfloat16, kind="ExternalInput")
    ow = nc.dram_tensor("ow", [128, NT * H], mybir.dt.bfloat16, kind="ExternalInput")
    dtw = nc.dram_tensor("dtw", [128, NT], mybir.dt.float32, kind="ExternalInput")
    wp0 = nc.dram_tensor("wp0", [H, H], mybir.dt.bfloat16, kind="ExternalInput")
    wp1 = nc.dram_tensor("wp1", [H, H], mybir.dt.bfloat16, kind="ExternalInput")
    wp2 = nc.dram_tensor("wp2", [H, H], mybir.dt.bfloat16, kind="ExternalInput")
    bps = nc.dram_tensor("bps", [1, 3 * H], mybir.dt.bfloat16, kind="ExternalInput")
    w2d = nc.dram_tensor("w2", [3 * H, C], mybir.dt.bfloat16, kind="ExternalInput")
    b2d = nc.dram_tensor("b2", [1, C], mybir.dt.bfloat16, kind="ExternalInput")
    idt = nc.dram_tensor("idt", [C, C], mybir.dt.bfloat16, kind="ExternalInput")
    id128 = nc.dram_tensor("id128", [128, 128], mybir.dt.bfloat16, kind="ExternalInput")
    lg_o = nc.dram_tensor("lg", [128, NT * C], mybir.dt.float32, kind="ExternalOutput")

    with tile.TileContext(nc) as tc:
        with (
            tc.tile_pool(name="per", bufs=1) as per,
            tc.tile_pool(name="sb", bufs=2) as sb,
            tc.tile_pool(name="ps", bufs=2, space="PSUM") as ps,
        ):
            get_blk = _make_stream(nc, sb, plan, vst, ohst)
            get_blk(0)  # prefetch first stream block before table loads
            ow_t = per.tile([128, NT, H], mybir.dt.bfloat16)
            dt = per.tile([128, NT], mybir.dt.float32)
            h2T_sb = per.tile([H, NTP], mybir.dt.bfloat16)
            wpt = [per.tile([H, H], mybir.dt.bfloat16, name=f"wpt{i}")
                   for i in range(3)]
            bps_t = per.tile([1, 3 * H], mybir.dt.bfloat16)
            w2t = [per.tile([H, C], mybir.dt.bfloat16, name=f"w2t{i}")
                   for i in range(3)]
            b2t = per.tile([1, C], mybir.dt.bfloat16)
            ones = per.tile([1, 512], mybir.dt.bfloat16)
            identC = per.tile([C, C], mybir.dt.bfloat16)
            ident128 = per.tile([128, 128], mybir.dt.bfloat16)
            nc.sync.dma_start(dt[:], dtw[:])
            nc.sync.dma_start(ow_t[:], ow.rearrange("p (t f) -> p t f", f=H))
            for i, wd in enumerate((wp0, wp1, wp2)):
                nc.sync.dma_start(wpt[i][:], wd[:])
                nc.sync.dma_start(w2t[i][:], w2d[i * H:(i + 1) * H, :])
            nc.sync.dma_start(bps_t[:], bps[:])
            nc.sync.dma_start(b2t[:], b2d[:])
            nc.sync.dma_start(identC[:], idt[:])
            nc.sync.dma_start(ident128[:], id128[:])
            nc.vector.memset(ones[:], 1.0)
            lgv = lg_o.rearrange("p (t f) -> p t f", f=C)

            def seg(t, hp):
                # h2 = dis*psum + ow ; transpose into h2T_sb column block
                cols = slice(t * 128, (t + 1) * 128)
                h2s = sb.tile([128, H], mybir.dt.bfloat16, tag="h2s", bufs=3,
                              name=f"h2s_{t}")
                nc.vector.scalar_tensor_tensor(
                    h2s[:], hp[:], dt[:, t:t + 1], ow_t[:, t, :],
                    ALU.mult, ALU.add)
                tp = ps.tile([H, 128], mybir.dt.bfloat16, tag="tp", bufs=1,
                             name=f"tp_{t}")
                nc.tensor.transpose(tp[:], h2s[:], ident128[:])
                nc.vector.tensor_copy(h2T_sb[:, cols], tp[:])

            def tail_block(t):
                if (t + 1) % TB != 0 and t != NT - 1:
                    return
                tb0 = (t // TB) * TB
                ntb = t + 1 - tb0
                W = ntb * 128
                cols = slice(tb0 * 128, tb0 * 128 + W)
                ht_b = sb.tile([H, 512], mybir.dt.bfloat16, tag="htb", bufs=2,
                               name=f"htb_{tb0}")
                h1t_b = sb.tile([H, 512], mybir.dt.bfloat16, tag="h1tb", bufs=2,
                                name=f"h1tb_{tb0}")
                nc.sync.dma_start(ht_b[:, :W], hT[:, cols])
                nc.sync.dma_start(h1t_b[:, :W], h1T[:, cols])
                XTs = (ht_b[:, :W], h1t_b[:, :W], h2T_sb[:, cols])
                z = sb.tile([H, 3, 512], mybir.dt.bfloat16, tag="z", bufs=2,
                            name=f"z_{tb0}")
                for i in range(3):
                    yb = ps.tile([H, 512], mybir.dt.float32, tag="yb", bufs=2,
                                 name=f"yb_{tb0}_{i}")
                    nc.tensor.matmul(yb[:, :W], wpt[i][:], XTs[i],
                                     start=True, stop=False)
                    nc.tensor.matmul(yb[:, :W], bps_t[:, i * H:(i + 1) * H],
                                     ones[:, :W], start=False, stop=True)
                    nc.scalar.activation(z[:, i, :W], yb[:, :W], AF.Relu)
                lt = ps.tile([C, 512], mybir.dt.float32, tag="lt", bufs=2,
                             name=f"lt_{tb0}")
                for i in range(3):
                    nc.tensor.matmul(lt[:, :W], w2t[i][:], z[:, i, :W],
                                     start=(i == 0), stop=False)
                nc.tensor.matmul(lt[:, :W], b2t[:], ones[:, :W],
                                 start=False, stop=True)
                lts = sb.tile([C, 512], mybir.dt.bfloat16, tag="lts", bufs=2,
                              name=f"lts_{tb0}")
                nc.vector.tensor_copy(lts[:, :W], lt[:, :W])
                lgb = sb.tile([128, TB, C], mybir.dt.float32, tag="lgb", bufs=2,
                              name=f"lgb_{tb0}")
                for j in range(ntb):
                    lgp = ps.tile([128, C], mybir.dt.bfloat16, tag="lgp", bufs=1,
                                  name=f"lgp_{tb0}_{j}")
                    nc.tensor.transpose(lgp[:], lts[:, j * 128:(j + 1) * 128],
                                        identC[:])
                    nc.vector.tensor_copy(lgb[:, j, :], lgp[:])
                # per-block log-softmax + output
                negm = sb.tile([128, TB, 1], mybir.dt.float32, tag="nm", bufs=2,
                               name=f"nm_{tb0}")
                xsb = sb.tile([128, TB, C], mybir.dt.float32, tag="xs", bufs=2,
                              name=f"xs_{tb0}")
                esb = sb.tile([128, TB, C], mybir.dt.float32, tag="es", bufs=2,
                              name=f"es_{tb0}")
                ssb = sb.tile([128, TB, 1], mybir.dt.float32, tag="ss", bufs=2,
                              name=f"ss_{tb0}")
                lsb = sb.tile([128, TB, 1], mybir.dt.float32, tag="ls", bufs=2,
                              name=f"ls_{tb0}")
                nb = slice(0, ntb)
                nc.vector.tensor_reduce(negm[:, nb, :], lgb[:, nb, :],
                                        mybir.AxisListType.X, ALU.max,
                                        negate=True)
                nc.vector.tensor_tensor(
                    xsb[:, nb, :], lgb[:, nb, :],
                    negm[:, nb, :].to_broadcast([128, ntb, C]), ALU.add)
                nc.scalar.activation(esb[:, nb, :], xsb[:, nb, :], AF.Exp)
                nc.vector.tensor_reduce(ssb[:, nb, :], esb[:, nb, :],
                                        mybir.AxisListType.X, ALU.add)
                nc.scalar.activation(lsb[:, nb, :], ssb[:, nb, :], AF.Ln)
                nc.vector.tensor_tensor(
                    xsb[:, nb, :], xsb[:, nb, :],
                    lsb[:, nb, :].to_broadcast([128, ntb, C]), ALU.subtract)
                nc.sync.dma_start(lgv[:, tb0:tb0 + ntb, :], xsb[:, nb, :])

            _hop_segs(nc, ps, plan, get_blk, seg, post_seg=tail_block)
    nc.compile()
    return nc


# --------------------------------------------------------------------------
# top-level entry
# --------------------------------------------------------------------------

def kernel(**inputs):
    x = np.asarray(inputs["x"], np.float32)
    edge_index = np.asarray(inputs["edge_index"])
    w1 = np.asarray(inputs["w1"], np.float32)
    b1 = np.asarray(inputs["b1"], np.float32)
    wps = [np.asarray(inputs[f"wp{i}"], np.float32) for i in range(3)]
    bps = [np.asarray(inputs[f"bp{i}"], np.float32) for i in range(3)]
    w2 = np.asarray(inputs["w2"], np.float32)
    b2 = np.asarray(inputs["b2"], np.float32)

    dis, srcs, ohs, plan = _prep_graph(edge_index)
    key = ("prog", plan["nkt_t"])
    if key not in _cache:
        _cache[key] = (_build_L1(), _build_L2(plan), _build_L3(plan))
    ncL1, ncL2, ncL3 = _cache[key]
    NKT = plan["NKT"]

    disw_c = [_wrap_tiles(dis[c * NSH:(c + 1) * NSH]) for c in range(NCORE)]

    # ---- L1
    in1 = []
    w1d = np.ascontiguousarray(
        w1.reshape(2, 128, H).transpose(1, 0, 2).reshape(128, 2 * H)).astype(FP8)
    for c in range(NCORE):
        xp = np.zeros((NTP, F_IN), np.float32)
        xp[:NSH] = x[c * NSH:(c + 1) * NSH]
        # xTd[p, t, s, q] = x[t*128+q, s*128+p]
        xTd = np.ascontiguousarray(
            xp.reshape(NT, 128, 2, 128).transpose(3, 0, 2, 1)
            .reshape(128, NT * 2 * 128)).astype(FP8)
        in1.append({"xTd": xTd, "w1d": w1d,
                    "b1r": b1[None, :].astype(BF16), "disw": disw_c[c]})
    _last_runs.clear()
    _last_runs.append(("L1", ncL1, in1))
    r1 = run_bass_kernel_spmd(ncL1, in1, list(range(NCORE)))
    h_c = [_unpm(r1.results[c]["h"], H).astype(np.float32)
           for c in range(NCORE)]
    u0f = np.concatenate([_unpm(r1.results[c]["u0"], H)
                          for c in range(NCORE)]).astype(np.float32)

    # ---- L2 (host materializes the u0[src] stream per core)
    in2 = []
    u0f8 = u0f.astype(FP8)
    for c in range(NCORE):
        dsh = dis[c * NSH:(c + 1) * NSH]
        u0own = u0f[c * NSH:(c + 1) * NSH]
        in2.append({
            "vst": _stream_pm(u0f8, srcs[c], NKT), "ohst": ohs[c],
            "ow1": _pm(dsh[:, None] * u0own).astype(BF16),
            "dtw": disw_c[c],
        })
    _last_runs.append(("L2", ncL2, in2))
    r2 = run_bass_kernel_spmd(ncL2, in2, list(range(NCORE)))
    h1_c = [_unpm(r2.results[c]["h1"], H).astype(np.float32)
            for c in range(NCORE)]
    u1f = np.concatenate([_unpm(r2.results[c]["u1"], H)
                          for c in range(NCORE)]).astype(np.float32)

    # ---- L3
    def padT(a):
        out = np.zeros((H, NTP), BF16)
        out[:, :a.shape[0]] = a.T.astype(BF16)
        return out

    bps_cat = np.concatenate(bps)[None, :].astype(BF16)
    u1f8 = u1f.astype(FP8)
    in3 = []
    for c in range(NCORE):
        dsh = dis[c * NSH:(c + 1) * NSH]
        u1own = u1f[c * NSH:(c + 1) * NSH]
        in3.append({
            "vst": _stream_pm(u1f8, srcs[c], NKT), "ohst": ohs[c],
            "hT": padT(h_c[c]), "h1T": padT(h1_c[c]),
            "ow": _pm(dsh[:, None] * u1own).astype(BF16),
            "dtw": disw_c[c],
            "wp0": wps[0].astype(BF16), "wp1": wps[1].astype(BF16),
            "wp2": wps[2].astype(BF16), "bps": bps_cat,
            "w2": w2.astype(BF16), "b2": b2[None, :].astype(BF16),
            "idt": np.eye(C, dtype=BF16),
            "id128": np.eye(128, dtype=BF16),
        })
    _last_runs.append(("L3", ncL3, in3))
    r3 = run_bass_kernel_spmd(ncL3, in3, list(range(NCORE)))
    out = np.concatenate([_unpm(r3.results[c]["lg"], C) for c in range(NCORE)])
    return out.astype(np.float32)
